# revision 32
# baseline (speedup 1.0000x reference)
"""Trainium2 Bass kernel for CustomMultiheadAttention.

Problem shapes: query/key/value [2048, 4, 1024] f32, causal mask [2048, 2048],
Wq/Wk/Wv/Wo [1024, 1024] (torch Linear layout [out, in]), biases [1024].
16 heads, head dim 64.

Sharding over 8 cores: core c -> (batch b = c // 2, head-group hg = c % 2).
Each core computes 8 heads (an E-slice of 512 rows of Wq/Wk/Wv, 512 cols of
Wo) for one batch. Host sums the two partial output projections per batch and
adds bo.

Device algorithm per core (bf16 matmuls, f32 PSUM accumulation):
  - Scores: per head pair (K=64 contraction at PE rows 0-63/64-127), emitted
    head-major (A-u0, A-u1, B-u0, B-u1) so exp(head A) never waits on head
    B's matmuls; diagonal s-blocks trim matmul/exp spans to the causal
    region, with the 128x128 boundary triangle masked by a 0/1 pattern.
  - exp on ScalarE batched [128, 2, 512] per group; PV matmuls [65, 512]
    (ones column accumulates the softmax denominator) run two groups behind
    the exp stream; one accumulation group per PSUM bank (HW clears the
    whole bank's has_written bits on start=True).
  - Normalize: 1/den = exp(-ln(den)) on ScalarE (ln+exp share one ACT table
    set; DVE reciprocal is ~6.5ns/elem serial on one lane), gpsimd
    partition_broadcast, one [64,512] DVE mul per head.
  - Q/K/V projections and the output projection are "filler units" popped
    into PE slack of the ACT-bound attention stream via a stride schedule;
    out-projection units are reserved for the last t-chunk and fillers pop
    in 2-unit bursts late (8 dense matmuls ~ one HAM warm-up window) to
    keep the PE clock at 8/8. All DMAs issue from sync/gpsimd queues so the
    scalar engine does nothing but activations.
"""

import math
import os
import sys

import numpy as np

for _p in ("/opt/trn_rl_repo", os.path.expanduser("~/.axon_site/_ro/trn_rl_repo")):
    if os.path.isdir(_p) and _p not in sys.path:
        sys.path.insert(0, _p)

import ml_dtypes  # noqa: E402

import concourse.bass as bass  # noqa: E402
import concourse.tile as tile  # noqa: E402
from concourse import bacc, bass_utils, library_config, mybir  # noqa: E402

# Problem constants
T, S, B, E, H = 2048, 2048, 4, 1024, 16
D = E // H  # 64
NCORES = 8
HC = H // 2  # heads per core
EH = HC * D  # 512 per-core E-slice
P = 128
TC = 512  # t-chunk
NT = T // TC  # 4
NSB = S // P  # 16 s-blocks
KO = E // P  # 8 contraction chunks for projections
KHD = EH // P  # 4 contraction chunks for out proj
VW = D + 1  # 65: head V width incl ones column
BF16 = mybir.dt.bfloat16
F32 = mybir.dt.float32
NPBF16 = ml_dtypes.bfloat16

_CACHE: dict = {}


def _build_nc():
    nc = bacc.Bacc(
        "TRN2",
        target_bir_lowering=False,
        debug=False,
        enable_asserts=True,
        num_devices=NCORES,
    )
    AF = mybir.ActivationFunctionType

    xq_t = nc.dram_tensor("xq_t", [E, T], BF16, kind="ExternalInput").ap()
    xk_t = nc.dram_tensor("xk_t", [E, T], BF16, kind="ExternalInput").ap()
    xv_t = nc.dram_tensor("xv_t", [E, T], BF16, kind="ExternalInput").ap()
    wq_t = nc.dram_tensor("wq_t", [E, EH], BF16, kind="ExternalInput").ap()
    wk_t = nc.dram_tensor("wk_t", [E, EH], BF16, kind="ExternalInput").ap()
    wv_t = nc.dram_tensor("wv_t", [E, EH], BF16, kind="ExternalInput").ap()
    wo_t = nc.dram_tensor("wo_t", [EH, E], BF16, kind="ExternalInput").ap()
    bq_d = nc.dram_tensor("bq_d", [P, KHD], F32, kind="ExternalInput").ap()
    bk_d = nc.dram_tensor("bk_d", [P, KHD], F32, kind="ExternalInput").ap()
    bv_d = nc.dram_tensor("bv_d", [P, EH], F32, kind="ExternalInput").ap()
    mask_d = nc.dram_tensor("mask_d", [P, 4, TC], BF16, kind="ExternalInput").ap()
    out_p = nc.dram_tensor("out_part", [T, E], BF16, kind="ExternalOutput").ap()

    from contextlib import ExitStack

    with tile.TileContext(nc) as tc, ExitStack() as ctx:
        persist = ctx.enter_context(tc.tile_pool(name="persist", bufs=1))
        xpool = ctx.enter_context(tc.tile_pool(name="x", bufs=6))
        exps = ctx.enter_context(tc.tile_pool(name="exps", bufs=8))
        evac = ctx.enter_context(tc.tile_pool(name="evac", bufs=6))
        norm = ctx.enter_context(tc.tile_pool(name="norm", bufs=4))
        psS = ctx.enter_context(tc.tile_pool(name="psS", bufs=2, space="PSUM"))
        psPV = ctx.enter_context(tc.tile_pool(name="psPV", bufs=2, space="PSUM"))
        psF = ctx.enter_context(tc.tile_pool(name="psF", bufs=2, space="PSUM"))

        nc.gpsimd.load_library(library_config.attn)  # for partition_broadcast

        # ---- persistent SBUF tiles
        wq_sb = persist.tile([P, KO, EH], BF16, tag="wq")
        wk_sb = persist.tile([P, KO, EH], BF16, tag="wk")
        wv_sb = persist.tile([P, KO, EH], BF16, tag="wv")
        bq_sb = persist.tile([P, KHD], F32, tag="bq")
        bk_sb = persist.tile([P, KHD], F32, tag="bk")
        bv_sb = persist.tile([P, EH], F32, tag="bv")
        mask_sb = persist.tile([P, 4, TC], BF16, tag="mask")
        wo_sb = persist.tile([P, KHD, E], BF16, tag="wo")
        qt_sb = persist.tile([P, KHD, T], BF16, tag="qt")
        kt_sb = persist.tile([P, KHD, T], BF16, tag="kt")
        v_sb = persist.tile([P, NSB, HC * VW], BF16, tag="v")
        attnT = persist.tile([P, KHD, T], BF16, tag="attnT")

        # ---- constant DMAs (vector + gpsimd queues; sync reserved for x)
        nc.gpsimd.dma_start(wk_sb[:], wk_t.rearrange("(ko p) m -> p ko m", p=P))
        nc.gpsimd.dma_start(wq_sb[:], wq_t.rearrange("(ko p) m -> p ko m", p=P))
        nc.gpsimd.dma_start(bq_sb[:], bq_d)
        nc.gpsimd.dma_start(bk_sb[:], bk_d)
        nc.gpsimd.dma_start(wv_sb[:], wv_t.rearrange("(ko p) m -> p ko m", p=P))
        nc.gpsimd.dma_start(bv_sb[:], bv_d)
        nc.gpsimd.dma_start(mask_sb[:], mask_d)
        for h in range(HC):
            nc.vector.memset(v_sb[:, :, h * VW + D : h * VW + VW], 1.0)

        # ---- x staging: one [P, KO, TC] chunk per (tensor, t-chunk)
        x_srcs = {
            "q": xq_t.rearrange("(ko p) t -> p ko t", p=P),
            "k": xk_t.rearrange("(ko p) t -> p ko t", p=P),
            "v": xv_t.rearrange("(ko p) t -> p ko t", p=P),
        }
        x_chunks: dict = {}
        held_v: list = []

        def dma_x(kind, cj):
            xt = xpool.tile([P, KO, TC], BF16, name="xt", tag="xt")
            nc.sync.dma_start(xt[:], x_srcs[kind][:, :, cj * TC : (cj + 1) * TC])
            x_chunks[(kind, cj)] = xt

        # ---- filler units (projections + out-projection), popped into PE slack
        def unit_qk(kind, db, cj):
            w_sb, b_sb, dst = (
                (wq_sb, bq_sb, qt_sb) if kind == "q" else (wk_sb, bk_sb, kt_sb)
            )

            def _emit():
                x_sb = x_chunks[(kind, cj)]
                ps = psF.tile([P, TC], F32, tag="pf")
                for ko in range(KO):
                    nc.tensor.matmul(
                        ps[:],
                        lhsT=w_sb[:, ko, db * P : (db + 1) * P],
                        rhs=x_sb[:, ko, :],
                        start=(ko == 0),
                        stop=(ko == KO - 1),
                    )
                nc.vector.tensor_scalar_add(
                    dst[:, db, cj * TC : (cj + 1) * TC], ps[:], b_sb[:, db : db + 1]
                )

            return _emit

        def unit_v(si):
            def _emit():
                x_sb = x_chunks[("v", si // 4)]
                r = si % 4
                ps = psF.tile([P, TC], F32, tag="pf")
                for ko in range(KO):
                    nc.tensor.matmul(
                        ps[:],
                        lhsT=x_sb[:, ko, r * P : (r + 1) * P],
                        rhs=wv_sb[:, ko, :],
                        start=(ko == 0),
                        stop=(ko == KO - 1),
                    )
                v_dst = v_sb[:, si, :].rearrange("p (h x) -> p h x", h=HC)[:, :, 0:D]
                nc.vector.tensor_add(
                    v_dst,
                    ps[:].rearrange("p (h x) -> p h x", h=HC),
                    bv_sb[:].rearrange("p (h x) -> p h x", h=HC),
                )

            return _emit

        def unit_outproj(tb):
            # both fj halves in one unit: each attnT lhsT is loaded once and
            # streamed twice; 8 dense matmuls ~ one HAM warm-up window
            def _emit():
                pos = [psF.tile([P, TC], F32, name="po", tag="pf") for _ in range(2)]
                for ko in range(KHD):
                    for fj in range(2):
                        nc.tensor.matmul(
                            pos[fj][:],
                            lhsT=attnT[:, ko, tb * P : (tb + 1) * P],
                            rhs=wo_sb[:, ko, fj * TC : (fj + 1) * TC],
                            start=(ko == 0),
                            stop=(ko == KHD - 1),
                        )
                for fj in range(2):
                    ot = evac.tile([P, TC], BF16, name="ot", tag="ot")
                    nc.vector.tensor_copy(ot[:], pos[fj][:])
                    nc.gpsimd.dma_start(
                        out_p[tb * P : (tb + 1) * P, fj * TC : (fj + 1) * TC], ot[:]
                    )

            return _emit

        pend_proj: list = []  # projection unit keys: JIT-critical for next tj
        pend_op: list = []  # out-projection units: reserved for the last tj
        unit_fns: dict = {}
        done_units: set = set()

        def reg(key, fn):
            unit_fns[key] = fn
            return key

        def run_unit(key):
            if key not in done_units:
                done_units.add(key)
                unit_fns[key]()

        def need(key):
            if key in unit_fns and key not in done_units:
                run_unit(key)
        sched = {"i": 0, "stride": 1, "use_op": False, "burst": False}

        def slot():
            # pop filler units every `stride` slots; in `burst` mode pop two
            # back-to-back (8 dense matmuls ~ a full HAM warm-up window)
            if sched["i"] % sched["stride"] == 0:
                for _ in range(2 if sched["burst"] else 1):
                    while pend_proj:
                        key = pend_proj.pop(0)
                        if key not in done_units:
                            run_unit(key)
                            break
                    else:
                        if sched["use_op"] and pend_op:
                            pend_op.pop(0)()
            sched["i"] += 1

        def drain_proj():
            for key in pend_proj:
                run_unit(key)
            pend_proj[:] = []

        # ---- startup: x for tj0 + minimal projection units, rest -> pending
        for kind in ("k", "q", "v"):
            dma_x(kind, 0)
        nc.gpsimd.dma_start(wo_sb[:], wo_t.rearrange("(ko p) m -> p ko m", p=P))
        for db in range(KHD):
            reg(("k", db, 0), unit_qk("k", db, 0))
            reg(("q", db, 0), unit_qk("q", db, 0))
        for si in range(4):
            reg(("v", si), unit_v(si))
        run_unit(("k", 0, 0))
        run_unit(("q", 0, 0))
        for kind in ("q", "k", "v"):
            dma_x(kind, 1)
        pend_proj += [("v", 0), ("v", 1), ("v", 2), ("v", 3)]
        pend_proj += [("k", 1, 0), ("q", 1, 0), ("k", 2, 0), ("q", 2, 0)]
        pend_proj += [("k", 3, 0), ("q", 3, 0)]

        # ---- attention: per t-chunk, 4 head-pairs (= kt/qt chunk ch)
        for tj in range(NT):
            ng = 2 * tj + 2
            si_last = 4 * tj + 3
            if tj >= 1:
                drain_proj()  # emission order defines deps: this tj's
                # projection units must precede its attention reads
            if tj < NT - 1:
                nxt = tj + 1
                for db in range(KHD):
                    reg(("k", db, nxt), unit_qk("k", db, nxt))
                    reg(("q", db, nxt), unit_qk("q", db, nxt))
                for si in range(4 * nxt, 4 * nxt + 4):
                    reg(("v", si), unit_v(si))
                vkeys = [("v", si) for si in range(4 * nxt, 4 * nxt + 4)]
                if nxt == NT - 1:
                    # hold the last chunk's V units back as tj3 filler: they
                    # are only read at its final groups (need() guards order)
                    held_v.extend(vkeys)
                    vkeys = []
                pend_proj.extend(
                    [("k", db, nxt) for db in range(KHD)]
                    + vkeys
                    + [("q", db, nxt) for db in range(KHD)]
                )
                if tj < NT - 2:
                    for kind in ("q", "k", "v"):
                        dma_x(kind, tj + 2)
            if tj >= 1:
                pend_op.extend(
                    [unit_outproj(tb) for tb in range(4 * (tj - 1), 4 * tj)]
                )
            if tj == NT - 1:
                pend_proj.extend(held_v)
                held_v[:] = []
            nslots = KHD * (2 * ng + 1)
            nunits = len(pend_proj) + (len(pend_op) if tj == NT - 1 else 0)
            sched["i"] = 0
            sched["burst"] = tj >= 1
            eff = 2 if tj >= 1 else 1
            sched["stride"] = max(1, (nslots * eff) // max(1, nunits))
            sched["use_op"] = tj == NT - 1

            for ch in range(KHD):  # head pair (2ch, 2ch+1)
                pvs = [psPV.tile([P, TC], F32, name="pv", tag="pv") for _ in range(2)]
                ets: dict = {}

                def emit_scores_exp(g, ch=ch, ets=ets):
                    need(("q", ch, tj))
                    for u in range(2):
                        need(("k", ch, (2 * g + u) // 4))
                    scs = [psS.tile([P, 2, TC], F32, name="sc", tag="sc") for _ in range(2)]
                    for hp in range(2):
                        pb = D * hp
                        for u in range(2):
                            si = 2 * g + u
                            f0 = max(0, P * (si - 4 * tj))
                            nc.tensor.matmul(
                                scs[hp][:, u, f0:TC],
                                lhsT=kt_sb[pb : pb + D, ch, si * P : (si + 1) * P],
                                rhs=qt_sb[pb : pb + D, ch, tj * TC + f0 : (tj + 1) * TC],
                                start=True,
                                stop=True,
                            )
                    et2 = []
                    for hp in range(2):
                        et = exps.tile([P, 2, TC], BF16, name="et", tag="et")
                        if g < 2 * tj:
                            nc.scalar.activation(
                                et[:], scs[hp][:], AF.Exp, scale=1.0 / math.sqrt(D)
                            )
                        else:
                            for u in range(2):
                                si = 2 * g + u
                                k = si - 4 * tj
                                f0 = P * k
                                nc.scalar.activation(
                                    et[:, u, f0:TC],
                                    scs[hp][:, u, f0:TC],
                                    AF.Exp,
                                    scale=1.0 / math.sqrt(D),
                                )
                                nc.vector.tensor_mul(
                                    et[:, u, f0 : f0 + P],
                                    et[:, u, f0 : f0 + P],
                                    mask_sb[:, k, f0 : f0 + P],
                                )
                        et2.append(et)
                    ets[g] = et2

                def emit_pv(g, ch=ch, ets=ets, pvs=pvs):
                    need(("v", 2 * g))
                    need(("v", 2 * g + 1))
                    et2 = ets.pop(g)
                    for u in range(2):
                        si = 2 * g + u
                        f0 = max(0, P * (si - 4 * tj))
                        for hp in range(2):
                            h = 2 * ch + hp  # core-head 0..7
                            nc.tensor.matmul(
                                pvs[hp][0:VW, f0:TC],
                                lhsT=v_sb[:, si, h * VW : (h + 1) * VW],
                                rhs=et2[hp][:, u, f0:TC],
                                start=(si == 0),
                                stop=(si == si_last),
                                skip_group_check=True,
                            )

                for g in range(ng):
                    emit_scores_exp(g)
                    slot()
                    if g >= 2:
                        emit_pv(g - 2)
                        slot()
                for gg in range(max(0, ng - 2), ng):
                    emit_pv(gg)
                    slot()

                # normalize pair: 1/den -> broadcast -> attnT
                for hp in range(2):
                    pb = D * hp
                    lnd = norm.tile([1, TC], F32, name="lnd", tag="lnd")
                    nc.scalar.activation(lnd[:], pvs[hp][D : D + 1, :], AF.Ln)
                    rec = norm.tile([1, TC], F32, name="rec", tag="rec")
                    nc.scalar.activation(rec[:], lnd[:], AF.Exp, scale=-1.0)
                    rbs = norm.tile([D, TC], F32, name="rbs", tag="rbs")
                    nc.gpsimd.partition_broadcast(rbs[:], rec[:])
                    nc.vector.tensor_mul(
                        attnT[pb : pb + D, ch, tj * TC : (tj + 1) * TC],
                        pvs[hp][0:D, :],
                        rbs[:],
                    )
                slot()

        # ---- tail: drain pending, then last t-chunk's out-projection
        drain_proj()
        while pend_op:
            pend_op.pop(0)()
        for tb in range(4 * (NT - 1), 4 * NT):
            unit_outproj(tb)()

    nc.compile()
    return nc


def _get_nc():
    if "nc" not in _CACHE:
        _CACHE["nc"] = _build_nc()
    return _CACHE["nc"]


def _prep_in_maps(query, key, value, attn_mask, Wq, bq, Wk, bk, Wv, bv, Wo, bo):
    """Host-side prep: slices, transposes, bf16 casts. Returns in_maps[8]."""
    f32 = np.float32
    xt = {}  # (kind, b) -> [E, T] bf16
    for b in range(B):
        xt[("q", b)] = np.ascontiguousarray(query[:, b, :].T).astype(NPBF16)
        xt[("k", b)] = np.ascontiguousarray(key[:, b, :].T).astype(NPBF16)
        xt[("v", b)] = np.ascontiguousarray(value[:, b, :].T).astype(NPBF16)
    wt = {}
    for hg in range(2):
        sl = slice(EH * hg, EH * hg + EH)
        wt[("q", hg)] = np.ascontiguousarray(Wq[sl, :].T).astype(NPBF16)
        wt[("k", hg)] = np.ascontiguousarray(Wk[sl, :].T).astype(NPBF16)
        wt[("v", hg)] = np.ascontiguousarray(Wv[sl, :].T).astype(NPBF16)
        wt[("o", hg)] = np.ascontiguousarray(Wo[:, sl].T).astype(NPBF16)
        wt[("bq", hg)] = np.ascontiguousarray(
            bq[sl].astype(f32).reshape(KHD, P).T
        )
        wt[("bk", hg)] = np.ascontiguousarray(
            bk[sl].astype(f32).reshape(KHD, P).T
        )
        wt[("bv", hg)] = np.ascontiguousarray(
            np.tile(bv[sl].astype(f32)[None, :], (P, 1))
        )
    # mask patterns: for a scores tile with s0 = t0 + 128*o, pattern
    # [p, o, f] = 0 if attn_mask[t0+f, s0+p] (masked) else 1.
    t0 = 512
    patts = []
    for o in range(4):
        s0 = t0 + P * o
        patts.append(
            (~np.asarray(attn_mask[t0 : t0 + TC, s0 : s0 + P])).T.astype(NPBF16)
        )
    mask_tiles = np.ascontiguousarray(np.stack(patts, axis=1))  # [P, 4, TC]

    in_maps = []
    for c in range(NCORES):
        b, hg = c // 2, c % 2
        in_maps.append(
            {
                "xq_t": xt[("q", b)],
                "xk_t": xt[("k", b)],
                "xv_t": xt[("v", b)],
                "wq_t": wt[("q", hg)],
                "wk_t": wt[("k", hg)],
                "wv_t": wt[("v", hg)],
                "wo_t": wt[("o", hg)],
                "bq_d": wt[("bq", hg)],
                "bk_d": wt[("bk", hg)],
                "bv_d": wt[("bv", hg)],
                "mask_d": mask_tiles,
            }
        )
    return in_maps


def _run_on_hw(in_maps, trace=False, **kwargs):
    nc = _get_nc()
    return bass_utils.run_bass_kernel_spmd(
        nc, in_maps, core_ids=list(range(NCORES)), trace=trace, **kwargs
    )


def _gather(results, bo):
    outs = []
    for b in range(B):
        part = np.asarray(results[2 * b]["out_part"], dtype=np.float32) + np.asarray(
            results[2 * b + 1]["out_part"], dtype=np.float32
        )
        outs.append(part)
    out = np.stack(outs, axis=1)  # [T, B, E]
    out += np.asarray(bo, dtype=np.float32)[None, None, :]
    return out.astype(np.float32)


def _numpy_fallback(query, key, value, attn_mask, Wq, bq, Wk, bk, Wv, bv, Wo, bo):
    """Exact f32 numpy replication of the reference (for non-causal masks)."""
    f32 = np.float32
    query, key, value = (np.asarray(a, f32) for a in (query, key, value))
    q = (np.einsum("tbe,fe->btf", query, Wq, dtype=f32) + bq).reshape(B, T, H, D)
    k = (np.einsum("sbe,fe->bsf", key, Wk, dtype=f32) + bk).reshape(B, S, H, D)
    v = (np.einsum("sbe,fe->bsf", value, Wv, dtype=f32) + bv).reshape(B, S, H, D)
    q, k, v = (a.transpose(0, 2, 1, 3) for a in (q, k, v))
    out = np.empty((B, H, T, D), f32)
    mask = np.asarray(attn_mask)
    for b in range(B):
        for h in range(H):
            sc = (q[b, h] @ k[b, h].T) / np.float32(math.sqrt(D))
            sc = np.where(mask, -np.inf, sc)
            m = np.max(sc, axis=-1, keepdims=True)
            m = np.where(np.isfinite(m), m, 0.0)
            e = np.exp(sc - m)
            p = e / np.sum(e, axis=-1, keepdims=True)
            p = np.where(np.isinf(sc), 0.0, p)
            out[b, h] = p @ v[b, h]
    out = out.transpose(0, 2, 1, 3).reshape(B, T, E)
    out = out @ np.asarray(Wo, f32).T + bo
    return np.ascontiguousarray(out.transpose(1, 0, 2)).astype(f32)


def kernel(query, key, value, attn_mask, Wq, bq, Wk, bk, Wv, bv, Wo, bo):
    mask = np.asarray(attn_mask)
    causal = mask.shape == (T, S) and np.array_equal(
        mask, np.triu(np.ones((T, S), dtype=bool), k=1)
    )
    if not causal:
        return _numpy_fallback(
            query, key, value, attn_mask, Wq, bq, Wk, bk, Wv, bv, Wo, bo
        )
    in_maps = _prep_in_maps(
        query, key, value, attn_mask, Wq, bq, Wk, bk, Wv, bv, Wo, bo
    )
    res = _run_on_hw(in_maps)
    return _gather(res.results, bo)


# revision 34
# speedup vs baseline: 1.0182x; 1.0182x over previous
"""Trainium2 Bass kernel for CustomMultiheadAttention.

Problem shapes: query/key/value [2048, 4, 1024] f32, causal mask [2048, 2048],
Wq/Wk/Wv/Wo [1024, 1024] (torch Linear layout [out, in]), biases [1024].
16 heads, head dim 64.

Sharding over 8 cores: core c -> (batch b = c // 2, head-group hg = c % 2).
Each core computes 8 heads (an E-slice of 512 rows of Wq/Wk/Wv, 512 cols of
Wo) for one batch. Host sums the two partial output projections per batch and
adds bo.

Device algorithm per core (bf16 matmuls, f32 PSUM accumulation):
  - Scores: per head pair (K=64 contraction at PE rows 0-63/64-127), emitted
    head-major (A-u0, A-u1, B-u0, B-u1) so exp(head A) never waits on head
    B's matmuls; diagonal s-blocks trim matmul/exp spans to the causal
    region, with the 128x128 boundary triangle masked by a 0/1 pattern.
  - exp on ScalarE batched [128, 2, 512] per group; PV matmuls [65, 512]
    (ones column accumulates the softmax denominator) run two groups behind
    the exp stream; one accumulation group per PSUM bank (HW clears the
    whole bank's has_written bits on start=True).
  - Normalize: 1/den = exp(-ln(den)) on ScalarE (ln+exp share one ACT table
    set; DVE reciprocal is ~6.5ns/elem serial on one lane), gpsimd
    partition_broadcast, one [64,512] DVE mul per head.
  - Q/K/V projections and the output projection are "filler units" popped
    into PE slack of the ACT-bound attention stream via a stride schedule;
    out-projection units are reserved for the last t-chunk and fillers pop
    in 2-unit bursts late (8 dense matmuls ~ one HAM warm-up window) to
    keep the PE clock at 8/8. All DMAs issue from sync/gpsimd queues so the
    scalar engine does nothing but activations.
"""

import math
import os
import sys

import numpy as np

for _p in ("/opt/trn_rl_repo", os.path.expanduser("~/.axon_site/_ro/trn_rl_repo")):
    if os.path.isdir(_p) and _p not in sys.path:
        sys.path.insert(0, _p)

import ml_dtypes  # noqa: E402

import concourse.bass as bass  # noqa: E402
import concourse.tile as tile  # noqa: E402
from concourse import bacc, bass_utils, library_config, mybir  # noqa: E402

# Problem constants
T, S, B, E, H = 2048, 2048, 4, 1024, 16
D = E // H  # 64
NCORES = 8
HC = H // 2  # heads per core
EH = HC * D  # 512 per-core E-slice
P = 128
TC = 512  # t-chunk
NT = T // TC  # 4
NSB = S // P  # 16 s-blocks
KO = E // P  # 8 contraction chunks for projections
KHD = EH // P  # 4 contraction chunks for out proj
VW = D + 1  # 65: head V width incl ones column
BF16 = mybir.dt.bfloat16
F32 = mybir.dt.float32
NPBF16 = ml_dtypes.bfloat16

_CACHE: dict = {}


def _build_nc():
    nc = bacc.Bacc(
        "TRN2",
        target_bir_lowering=False,
        debug=False,
        enable_asserts=True,
        num_devices=NCORES,
    )
    AF = mybir.ActivationFunctionType

    xq_t = nc.dram_tensor("xq_t", [E, T], BF16, kind="ExternalInput").ap()
    xk_t = nc.dram_tensor("xk_t", [E, T], BF16, kind="ExternalInput").ap()
    xv_t = nc.dram_tensor("xv_t", [E, T], BF16, kind="ExternalInput").ap()
    wq_t = nc.dram_tensor("wq_t", [E, EH], BF16, kind="ExternalInput").ap()
    wk_t = nc.dram_tensor("wk_t", [E, EH], BF16, kind="ExternalInput").ap()
    wv_t = nc.dram_tensor("wv_t", [E, EH], BF16, kind="ExternalInput").ap()
    wo_t = nc.dram_tensor("wo_t", [EH, E], BF16, kind="ExternalInput").ap()
    bq_d = nc.dram_tensor("bq_d", [P, KHD], F32, kind="ExternalInput").ap()
    bk_d = nc.dram_tensor("bk_d", [P, KHD], F32, kind="ExternalInput").ap()
    bv_d = nc.dram_tensor("bv_d", [P, EH], F32, kind="ExternalInput").ap()
    mask_d = nc.dram_tensor("mask_d", [P, 4, TC], BF16, kind="ExternalInput").ap()
    out_p = nc.dram_tensor("out_part", [T, E], BF16, kind="ExternalOutput").ap()

    from contextlib import ExitStack

    with tile.TileContext(nc) as tc, ExitStack() as ctx:
        persist = ctx.enter_context(tc.tile_pool(name="persist", bufs=1))
        xpool = ctx.enter_context(tc.tile_pool(name="x", bufs=6))
        exps = ctx.enter_context(tc.tile_pool(name="exps", bufs=8))
        evac = ctx.enter_context(tc.tile_pool(name="evac", bufs=6))
        norm = ctx.enter_context(tc.tile_pool(name="norm", bufs=4))
        psS = ctx.enter_context(tc.tile_pool(name="psS", bufs=2, space="PSUM"))
        psPV = ctx.enter_context(tc.tile_pool(name="psPV", bufs=2, space="PSUM"))
        psF = ctx.enter_context(tc.tile_pool(name="psF", bufs=2, space="PSUM"))

        nc.gpsimd.load_library(library_config.attn)  # for partition_broadcast

        # ---- persistent SBUF tiles
        wq_sb = persist.tile([P, KO, EH], BF16, tag="wq")
        wk_sb = persist.tile([P, KO, EH], BF16, tag="wk")
        wv_sb = persist.tile([P, KO, EH], BF16, tag="wv")
        bq_sb = persist.tile([P, KHD], F32, tag="bq")
        bk_sb = persist.tile([P, KHD], F32, tag="bk")
        bv_sb = persist.tile([P, EH], F32, tag="bv")
        mask_sb = persist.tile([P, 4, TC], BF16, tag="mask")
        wo_sb = persist.tile([P, KHD, E], BF16, tag="wo")
        qt_sb = persist.tile([P, KHD, T], BF16, tag="qt")
        kt_sb = persist.tile([P, KHD, T], BF16, tag="kt")
        v_sb = persist.tile([P, NSB, HC * VW], BF16, tag="v")
        attnT = persist.tile([P, KHD, T], BF16, tag="attnT")

        # ---- constant DMAs (vector + gpsimd queues; sync reserved for x)
        nc.gpsimd.dma_start(wk_sb[:], wk_t.rearrange("(ko p) m -> p ko m", p=P))
        nc.gpsimd.dma_start(wq_sb[:], wq_t.rearrange("(ko p) m -> p ko m", p=P))
        nc.gpsimd.dma_start(bq_sb[:], bq_d)
        nc.gpsimd.dma_start(bk_sb[:], bk_d)
        nc.gpsimd.dma_start(wv_sb[:], wv_t.rearrange("(ko p) m -> p ko m", p=P))
        nc.gpsimd.dma_start(bv_sb[:], bv_d)
        nc.gpsimd.dma_start(mask_sb[:], mask_d)
        for h in range(HC):
            nc.vector.memset(v_sb[:, :, h * VW + D : h * VW + VW], 1.0)

        # ---- x staging: one [P, KO, TC] chunk per (tensor, t-chunk)
        x_srcs = {
            "q": xq_t.rearrange("(ko p) t -> p ko t", p=P),
            "k": xk_t.rearrange("(ko p) t -> p ko t", p=P),
            "v": xv_t.rearrange("(ko p) t -> p ko t", p=P),
        }
        x_chunks: dict = {}

        def dma_x(kind, cj):
            xt = xpool.tile([P, KO, TC], BF16, name="xt", tag="xt")
            nc.sync.dma_start(xt[:], x_srcs[kind][:, :, cj * TC : (cj + 1) * TC])
            x_chunks[(kind, cj)] = xt

        # ---- filler units (projections + out-projection), popped into PE slack
        def unit_qk(kind, db, cj):
            w_sb, b_sb, dst = (
                (wq_sb, bq_sb, qt_sb) if kind == "q" else (wk_sb, bk_sb, kt_sb)
            )

            def _emit():
                x_sb = x_chunks[(kind, cj)]
                ps = psF.tile([P, TC], F32, tag="pf")
                for ko in range(KO):
                    nc.tensor.matmul(
                        ps[:],
                        lhsT=w_sb[:, ko, db * P : (db + 1) * P],
                        rhs=x_sb[:, ko, :],
                        start=(ko == 0),
                        stop=(ko == KO - 1),
                    )
                nc.vector.tensor_scalar_add(
                    dst[:, db, cj * TC : (cj + 1) * TC], ps[:], b_sb[:, db : db + 1]
                )

            return _emit

        def unit_v(si):
            def _emit():
                x_sb = x_chunks[("v", si // 4)]
                r = si % 4
                ps = psF.tile([P, TC], F32, tag="pf")
                for ko in range(KO):
                    nc.tensor.matmul(
                        ps[:],
                        lhsT=x_sb[:, ko, r * P : (r + 1) * P],
                        rhs=wv_sb[:, ko, :],
                        start=(ko == 0),
                        stop=(ko == KO - 1),
                    )
                v_dst = v_sb[:, si, :].rearrange("p (h x) -> p h x", h=HC)[:, :, 0:D]
                nc.vector.tensor_add(
                    v_dst,
                    ps[:].rearrange("p (h x) -> p h x", h=HC),
                    bv_sb[:].rearrange("p (h x) -> p h x", h=HC),
                )

            return _emit

        def unit_outproj(tb):
            # both fj halves in one unit: each attnT lhsT is loaded once and
            # streamed twice; 8 dense matmuls ~ one HAM warm-up window
            def _emit():
                pos = [psF.tile([P, TC], F32, name="po", tag="pf") for _ in range(2)]
                for ko in range(KHD):
                    for fj in range(2):
                        nc.tensor.matmul(
                            pos[fj][:],
                            lhsT=attnT[:, ko, tb * P : (tb + 1) * P],
                            rhs=wo_sb[:, ko, fj * TC : (fj + 1) * TC],
                            start=(ko == 0),
                            stop=(ko == KHD - 1),
                        )
                for fj in range(2):
                    ot = evac.tile([P, TC], BF16, name="ot", tag="ot")
                    nc.vector.tensor_copy(ot[:], pos[fj][:])
                    nc.sync.dma_start(
                        out_p[tb * P : (tb + 1) * P, fj * TC : (fj + 1) * TC], ot[:]
                    )

            return _emit

        pend_proj: list = []  # projection unit keys: JIT-critical for next tj
        pend_op: list = []  # out-projection units: reserved for the last tj
        unit_fns: dict = {}
        done_units: set = set()

        def reg(key, fn):
            unit_fns[key] = fn
            return key

        def run_unit(key):
            if key not in done_units:
                done_units.add(key)
                unit_fns[key]()

        def need(key):
            if key in unit_fns and key not in done_units:
                run_unit(key)
        sched = {"i": 0, "stride": 1, "use_op": False, "burst": False}

        def slot():
            # pop filler units every `stride` slots; in `burst` mode pop two
            # back-to-back (8 dense matmuls ~ a full HAM warm-up window)
            if sched["i"] % sched["stride"] == 0:
                for _ in range(2 if sched["burst"] else 1):
                    while pend_proj:
                        key = pend_proj.pop(0)
                        if key not in done_units:
                            run_unit(key)
                            break
                    else:
                        if sched["use_op"] and pend_op:
                            pend_op.pop(0)()
            sched["i"] += 1

        def drain_proj():
            for key in pend_proj:
                run_unit(key)
            pend_proj[:] = []

        # ---- startup: x for tj0 + minimal projection units, rest -> pending
        for kind in ("k", "q", "v"):
            dma_x(kind, 0)
        nc.gpsimd.dma_start(wo_sb[:], wo_t.rearrange("(ko p) m -> p ko m", p=P))
        for db in range(KHD):
            reg(("k", db, 0), unit_qk("k", db, 0))
            reg(("q", db, 0), unit_qk("q", db, 0))
        for si in range(4):
            reg(("v", si), unit_v(si))
        run_unit(("k", 0, 0))
        run_unit(("q", 0, 0))
        for kind in ("q", "k", "v"):
            dma_x(kind, 1)
        pend_proj += [("v", 0), ("v", 1), ("v", 2), ("v", 3)]
        pend_proj += [("k", 1, 0), ("q", 1, 0), ("k", 2, 0), ("q", 2, 0)]
        pend_proj += [("k", 3, 0), ("q", 3, 0)]

        # ---- attention: per t-chunk, 4 head-pairs (= kt/qt chunk ch)
        for tj in range(NT):
            ng = 2 * tj + 2
            si_last = 4 * tj + 3
            if tj >= 1:
                drain_proj()  # emission order defines deps: this tj's
                # projection units must precede its attention reads
            if tj < NT - 1:
                nxt = tj + 1
                for db in range(KHD):
                    reg(("k", db, nxt), unit_qk("k", db, nxt))
                    reg(("q", db, nxt), unit_qk("q", db, nxt))
                for si in range(4 * nxt, 4 * nxt + 4):
                    reg(("v", si), unit_v(si))
                pend_proj.extend(
                    [("k", db, nxt) for db in range(KHD)]
                    + [("v", si) for si in range(4 * nxt, 4 * nxt + 4)]
                    + [("q", db, nxt) for db in range(KHD)]
                )
                if tj < NT - 2:
                    for kind in ("q", "k", "v"):
                        dma_x(kind, tj + 2)
            if tj >= 1:
                pend_op.extend(
                    [unit_outproj(tb) for tb in range(4 * (tj - 1), 4 * tj)]
                )
            nslots = KHD * (2 * ng + 1)
            nunits = len(pend_proj) + (len(pend_op) if tj == NT - 1 else 0)
            sched["i"] = 0
            sched["burst"] = tj >= 1
            eff = 2 if tj >= 1 else 1
            sched["stride"] = max(1, (nslots * eff) // max(1, nunits))
            sched["use_op"] = tj == NT - 1

            for ch in range(KHD):  # head pair (2ch, 2ch+1)
                pvs = [psPV.tile([P, TC], F32, name="pv", tag="pv") for _ in range(2)]
                ets: dict = {}

                def emit_scores_exp(g, ch=ch, ets=ets):
                    need(("q", ch, tj))
                    for u in range(2):
                        need(("k", ch, (2 * g + u) // 4))
                    scs = [psS.tile([P, 2, TC], F32, name="sc", tag="sc") for _ in range(2)]
                    for hp in range(2):
                        pb = D * hp
                        for u in range(2):
                            si = 2 * g + u
                            f0 = max(0, P * (si - 4 * tj))
                            nc.tensor.matmul(
                                scs[hp][:, u, f0:TC],
                                lhsT=kt_sb[pb : pb + D, ch, si * P : (si + 1) * P],
                                rhs=qt_sb[pb : pb + D, ch, tj * TC + f0 : (tj + 1) * TC],
                                start=True,
                                stop=True,
                            )
                    et2 = []
                    for hp in range(2):
                        et = exps.tile([P, 2, TC], BF16, name="et", tag="et")
                        if g < 2 * tj:
                            nc.scalar.activation(
                                et[:], scs[hp][:], AF.Exp, scale=1.0 / math.sqrt(D)
                            )
                        else:
                            for u in range(2):
                                si = 2 * g + u
                                k = si - 4 * tj
                                f0 = P * k
                                nc.scalar.activation(
                                    et[:, u, f0:TC],
                                    scs[hp][:, u, f0:TC],
                                    AF.Exp,
                                    scale=1.0 / math.sqrt(D),
                                )
                                nc.vector.tensor_mul(
                                    et[:, u, f0 : f0 + P],
                                    et[:, u, f0 : f0 + P],
                                    mask_sb[:, k, f0 : f0 + P],
                                )
                        et2.append(et)
                    ets[g] = et2

                def emit_pv(g, ch=ch, ets=ets, pvs=pvs):
                    need(("v", 2 * g))
                    need(("v", 2 * g + 1))
                    et2 = ets.pop(g)
                    for u in range(2):
                        si = 2 * g + u
                        f0 = max(0, P * (si - 4 * tj))
                        for hp in range(2):
                            h = 2 * ch + hp  # core-head 0..7
                            nc.tensor.matmul(
                                pvs[hp][0:VW, f0:TC],
                                lhsT=v_sb[:, si, h * VW : (h + 1) * VW],
                                rhs=et2[hp][:, u, f0:TC],
                                start=(si == 0),
                                stop=(si == si_last),
                                skip_group_check=True,
                            )

                for g in range(ng):
                    emit_scores_exp(g)
                    slot()
                    if g >= 2:
                        emit_pv(g - 2)
                        slot()
                for gg in range(max(0, ng - 2), ng):
                    emit_pv(gg)
                    slot()

                # normalize pair: 1/den -> broadcast -> attnT
                for hp in range(2):
                    pb = D * hp
                    lnd = norm.tile([1, TC], F32, name="lnd", tag="lnd")
                    nc.scalar.activation(lnd[:], pvs[hp][D : D + 1, :], AF.Ln)
                    rec = norm.tile([1, TC], F32, name="rec", tag="rec")
                    nc.scalar.activation(rec[:], lnd[:], AF.Exp, scale=-1.0)
                    rbs = norm.tile([D, TC], F32, name="rbs", tag="rbs")
                    nc.gpsimd.partition_broadcast(rbs[:], rec[:])
                    nc.vector.tensor_mul(
                        attnT[pb : pb + D, ch, tj * TC : (tj + 1) * TC],
                        pvs[hp][0:D, :],
                        rbs[:],
                    )
                slot()

        # ---- tail: drain pending, then last t-chunk's out-projection
        drain_proj()
        while pend_op:
            pend_op.pop(0)()
        for tb in range(4 * (NT - 1), 4 * NT):
            unit_outproj(tb)()

    nc.compile()
    return nc


def _get_nc():
    if "nc" not in _CACHE:
        _CACHE["nc"] = _build_nc()
    return _CACHE["nc"]


def _prep_in_maps(query, key, value, attn_mask, Wq, bq, Wk, bk, Wv, bv, Wo, bo):
    """Host-side prep: slices, transposes, bf16 casts. Returns in_maps[8]."""
    f32 = np.float32
    xt = {}  # (kind, b) -> [E, T] bf16
    for b in range(B):
        xt[("q", b)] = np.ascontiguousarray(query[:, b, :].T).astype(NPBF16)
        xt[("k", b)] = np.ascontiguousarray(key[:, b, :].T).astype(NPBF16)
        xt[("v", b)] = np.ascontiguousarray(value[:, b, :].T).astype(NPBF16)
    wt = {}
    for hg in range(2):
        sl = slice(EH * hg, EH * hg + EH)
        wt[("q", hg)] = np.ascontiguousarray(Wq[sl, :].T).astype(NPBF16)
        wt[("k", hg)] = np.ascontiguousarray(Wk[sl, :].T).astype(NPBF16)
        wt[("v", hg)] = np.ascontiguousarray(Wv[sl, :].T).astype(NPBF16)
        wt[("o", hg)] = np.ascontiguousarray(Wo[:, sl].T).astype(NPBF16)
        wt[("bq", hg)] = np.ascontiguousarray(
            bq[sl].astype(f32).reshape(KHD, P).T
        )
        wt[("bk", hg)] = np.ascontiguousarray(
            bk[sl].astype(f32).reshape(KHD, P).T
        )
        wt[("bv", hg)] = np.ascontiguousarray(
            np.tile(bv[sl].astype(f32)[None, :], (P, 1))
        )
    # mask patterns: for a scores tile with s0 = t0 + 128*o, pattern
    # [p, o, f] = 0 if attn_mask[t0+f, s0+p] (masked) else 1.
    t0 = 512
    patts = []
    for o in range(4):
        s0 = t0 + P * o
        patts.append(
            (~np.asarray(attn_mask[t0 : t0 + TC, s0 : s0 + P])).T.astype(NPBF16)
        )
    mask_tiles = np.ascontiguousarray(np.stack(patts, axis=1))  # [P, 4, TC]

    in_maps = []
    for c in range(NCORES):
        b, hg = c // 2, c % 2
        in_maps.append(
            {
                "xq_t": xt[("q", b)],
                "xk_t": xt[("k", b)],
                "xv_t": xt[("v", b)],
                "wq_t": wt[("q", hg)],
                "wk_t": wt[("k", hg)],
                "wv_t": wt[("v", hg)],
                "wo_t": wt[("o", hg)],
                "bq_d": wt[("bq", hg)],
                "bk_d": wt[("bk", hg)],
                "bv_d": wt[("bv", hg)],
                "mask_d": mask_tiles,
            }
        )
    return in_maps


def _run_on_hw(in_maps, trace=False, **kwargs):
    nc = _get_nc()
    return bass_utils.run_bass_kernel_spmd(
        nc, in_maps, core_ids=list(range(NCORES)), trace=trace, **kwargs
    )


def _gather(results, bo):
    outs = []
    for b in range(B):
        part = np.asarray(results[2 * b]["out_part"], dtype=np.float32) + np.asarray(
            results[2 * b + 1]["out_part"], dtype=np.float32
        )
        outs.append(part)
    out = np.stack(outs, axis=1)  # [T, B, E]
    out += np.asarray(bo, dtype=np.float32)[None, None, :]
    return out.astype(np.float32)


def _numpy_fallback(query, key, value, attn_mask, Wq, bq, Wk, bk, Wv, bv, Wo, bo):
    """Exact f32 numpy replication of the reference (for non-causal masks)."""
    f32 = np.float32
    query, key, value = (np.asarray(a, f32) for a in (query, key, value))
    q = (np.einsum("tbe,fe->btf", query, Wq, dtype=f32) + bq).reshape(B, T, H, D)
    k = (np.einsum("sbe,fe->bsf", key, Wk, dtype=f32) + bk).reshape(B, S, H, D)
    v = (np.einsum("sbe,fe->bsf", value, Wv, dtype=f32) + bv).reshape(B, S, H, D)
    q, k, v = (a.transpose(0, 2, 1, 3) for a in (q, k, v))
    out = np.empty((B, H, T, D), f32)
    mask = np.asarray(attn_mask)
    for b in range(B):
        for h in range(H):
            sc = (q[b, h] @ k[b, h].T) / np.float32(math.sqrt(D))
            sc = np.where(mask, -np.inf, sc)
            m = np.max(sc, axis=-1, keepdims=True)
            m = np.where(np.isfinite(m), m, 0.0)
            e = np.exp(sc - m)
            p = e / np.sum(e, axis=-1, keepdims=True)
            p = np.where(np.isinf(sc), 0.0, p)
            out[b, h] = p @ v[b, h]
    out = out.transpose(0, 2, 1, 3).reshape(B, T, E)
    out = out @ np.asarray(Wo, f32).T + bo
    return np.ascontiguousarray(out.transpose(1, 0, 2)).astype(f32)


def kernel(query, key, value, attn_mask, Wq, bq, Wk, bk, Wv, bv, Wo, bo):
    mask = np.asarray(attn_mask)
    causal = mask.shape == (T, S) and np.array_equal(
        mask, np.triu(np.ones((T, S), dtype=bool), k=1)
    )
    if not causal:
        return _numpy_fallback(
            query, key, value, attn_mask, Wq, bq, Wk, bk, Wv, bv, Wo, bo
        )
    in_maps = _prep_in_maps(
        query, key, value, attn_mask, Wq, bq, Wk, bk, Wv, bv, Wo, bo
    )
    res = _run_on_hw(in_maps)
    return _gather(res.results, bo)
